# revision 1
# baseline (speedup 1.0000x reference)
"""GATv2Stack Trainium2 kernel (8-core data-parallel over graphs), v2.

bt=128 graphs of N=64 nodes, 16 graphs/core. See reference.py.
  h = x @ W_in + b_in
  2x: xl=h@Wl+bl; xr=h@Wr+br; e=att.lrelu(xr_i+xl_j); a=softmax_j(e+mask)
      g = a@(h@Wl) + (out_bias+bl); g=ELU(g); g=LN(g); h=g+h; h=mask*h
  out = where(keep_graph, h, x@W_in+b_in)

v2 design (tuned for real-HW fixed per-instruction costs):
  - x pre-transposed on host (d-major), everything fp16 on device
  - no PE transposes: all layout flips via DMA XBAR transpose (16-bit)
  - e' = 0.8*att.max(-xl_j, xr_i) + (att.xl)_j   [0.2*att.xr_i const in j]
  - e_sb cols (par, i, hp, j) -> merged scatter (32 DMAs/layer) into wide
    et_w [(par,i), (gp, h, j)]; softmax wide; no rz mult: 1/Z folded into
    the psum->gn copy is wrong per-h, so rz applied as one wide DVE op
  - attention out: alpha^T slabs (dma-transposed) as STATIONARY, xl node
    -major (with ob+bl folded) as moving -> node-major out2, no post-
    attention transposes; LN centering on Act engine per-gp slices
  - elementwise work split DVE/GpSimd; psum evac via Act (Identity+bias)
Per-core layouts (G=16 graphs, gp pair idx, par=g%2):
  hT[m]     [128,1024] f16  [m*128+c, g*64+node]
  h_node    [128,2048] f16  [par*64+node, gp*256+ch]
  xlTb/xlTn/xrTb/xlOb[hp] [128,1024] f16 (t,c) x (g,node)
  sl (gp,hp) [128,2*m*m]  f16 cols par*m*m + i*m + j
  e_sb (gp)  [128,4*m*m]  f16 rows {0,1}=t, cols par*2mm + i*2m + hp*m + j
  et_w      [128,2048] f16  [par*64+i, gp*256 + h*64 + j] = exp(e'-4)
  aT_w      [128,2048] f16  [t*64+j, (2gp+hp)*128 + par*64 + i]
"""
import sys
sys.path.insert(0, '/opt/trn_rl_repo')
import numpy as np

import concourse.bass as bass
import concourse.mybir as mybir
from concourse import bass_utils, bacc
from concourse.tile import TileContext

dt = mybir.dt
F32, F16 = dt.float32, dt.float16
AF = mybir.ActivationFunctionType
ALU = mybir.AluOpType

B, T, N, D_IN = 2, 64, 64, 512
HID, L, H, C = 256, 2, 4, 64
BT = B * T
G = 16
NCORES = 8
LN_EPS = 1e-5
NEG_BIG = -30000.0

_n = [0]
def _nm(p="t"):
    _n[0] += 1
    return f"{p}{_n[0]}"


def fd(ap, *dims):
    """Keep partition dim + offset of (sliced) AP, replace free dims."""
    return bass.AP(ap.tensor, ap.offset, [list(ap.ap[0])] + [[s, c] for (s, c) in dims])


def _chunking(m):
    """Uniform i-chunks: smallest even nch with (m/nch)*m <= 512."""
    nch = 2
    while (m // nch) * m > 512 or m % nch != 0:
        nch += 2
    return nch, m // nch


def build_nc(mh=(64,) * G):
    nc = bacc.Bacc("TRN2", target_bir_lowering=False, debug=False,
                   enable_asserts=False, num_devices=1)

    def din(name, shape, dtp=F16):
        return nc.dram_tensor(name, list(shape), dtp, kind="ExternalInput").ap()

    xT_d    = din("xT", [D_IN, G * 64])
    win_d   = din("w_in", [D_IN, HID])
    wl_d    = din("wl", [L, HID, HID])
    wr_d    = din("wr", [L, HID, HID])
    binT_d  = din("binT", [128, 2], F32)
    blT_d   = din("blT", [128, 2 * L], F32)
    nblT_d  = din("nblT", [128, 2 * L], F32)
    oblT_d  = din("oblT", [128, 2 * L], F32)
    brT_d   = din("brT", [128, 2 * L], F32)
    att_d   = din("attBD", [128, 32 * 2 * L], F32)
    gam_d   = din("gam_f", [L, 128, HID])
    bet_d   = din("bet_f", [L, 128, HID])
    maskj_d = din("maskj_w", [128, 8 * 64], F32)
    mvec_d  = din("mvec_w", [128, 8])
    out_d   = nc.dram_tensor("out", [G * 64, HID], F16, kind="ExternalOutput").ap()

    mmax = max(mh)
    big_m = mmax > 48
    dmac = [0]
    def dmae():
        dmac[0] += 1
        return nc.sync if dmac[0] % 2 == 0 else nc.scalar
    vecc = [0]
    def vece():
        vecc[0] += 1
        return nc.vector if vecc[0] % 2 == 0 else nc.gpsimd

    with TileContext(nc) as tc:
        with tc.tile_pool(name="const", bufs=1) as cpool, \
             tc.tile_pool(name="wide", bufs=1) as wpool, \
             tc.tile_pool(name="slp", bufs=1 if big_m else 2) as slpool, \
             tc.tile_pool(name="sm", bufs=2) as smpool, \
             tc.tile_pool(name="psum", bufs=1, space="PSUM") as ppool:

            def ctile(name, dram_ap, shape, dtp=F16):
                t0 = cpool.tile(shape, dtp, name=_nm(name))
                nc.sync.dma_start(t0[:], dram_ap)
                return t0

            win_r = win_d.rearrange("(k p) n -> k p n", p=128)
            win = [ctile(f"win{k}", win_r[k], [128, HID]) for k in range(4)]
            wl, wr = [], []
            for l in range(L):
                wl_r = wl_d[l].rearrange("(k p) n -> k p n", p=128)
                wr_r = wr_d[l].rearrange("(k p) n -> k p n", p=128)
                wl.append([ctile(f"wl{l}{k}", wl_r[k], [128, HID]) for k in range(2)])
                wr.append([ctile(f"wr{l}{k}", wr_r[k], [128, HID]) for k in range(2)])
            binT  = ctile("binT", binT_d, [128, 2], F32)
            blT   = ctile("blT", blT_d, [128, 2 * L], F32)
            nblT  = ctile("nblT", nblT_d, [128, 2 * L], F32)
            oblT  = ctile("oblT", oblT_d, [128, 2 * L], F32)
            brT   = ctile("brT", brT_d, [128, 2 * L], F32)
            attBD_f32 = ctile("attBD", att_d, [128, 32 * 2 * L], F32)
            att08 = cpool.tile([128, 32 * 2 * L], F16, name=_nm("att08"))
            nc.vector.tensor_scalar(att08[:], attBD_f32[:], 0.8, None, op0=ALU.mult)
            att10 = cpool.tile([128, 32 * 2 * L], F16, name=_nm("att10"))
            nc.vector.tensor_copy(att10[:], attBD_f32[:])
            gam   = [ctile(f"gam{l}", gam_d[l], [128, HID]) for l in range(L)]
            bet   = [ctile(f"bet{l}", bet_d[l], [128, HID]) for l in range(L)]
            maskj = ctile("maskj", maskj_d, [128, 8 * 64], F32)
            mvec  = ctile("mvec", mvec_d, [128, 8])
            nbias = cpool.tile([128, 1], F32, name=_nm("nbias"))
            nc.vector.memset(nbias[:], -4.0)
            epsb = cpool.tile([128, 1], F32, name=_nm("epsb"))
            nc.vector.memset(epsb[:], LN_EPS)

            # ---------- input: load xT (d-major), project ----------
            hT = [smpool.tile([128, G * 64], F16, name=_nm("hT"), tag=f"hT{m}", bufs=1)
                  for m in range(2)]
            with tc.tile_pool(name="xtp", bufs=1) as xtpool:
                xT = [xtpool.tile([128, G * 64], F16, name=_nm("xT")) for _ in range(4)]
                xT_r = xT_d.rearrange("(k p) n -> k p n", p=128)
                for k in range(4):
                    for hh in range(2):
                        dmae().dma_start(xT[k][:, hh * 512:(hh + 1) * 512],
                                         xT_r[k][:, hh * 512:(hh + 1) * 512])
                for m in range(2):
                    for cb in range(2):
                        ph = ppool.tile([128, 512], F32, name=_nm("ph"), tag="pps", bufs=1)
                        for k in range(4):
                            nc.tensor.matmul(ph[:], win[k][:, m * 128:(m + 1) * 128],
                                             xT[k][:, cb * 512:(cb + 1) * 512],
                                             start=(k == 0), stop=(k == 3))
                        nc.scalar.activation(hT[m][:, cb * 512:(cb + 1) * 512], ph[:],
                                             AF.Identity, bias=binT[:, m:m + 1])

            h_node_w = smpool.tile([128, 8 * HID], F16, name=_nm("hnode"), tag="hnode",
                                   bufs=2)
            for gp in range(8):
                for m in range(2):
                    dmae().dma_start_transpose(
                        h_node_w[:, gp * HID + m * 128:gp * HID + m * 128 + 128],
                        hT[m][:, gp * 128:(gp + 1) * 128])

            # ---------- layers ----------
            # etT[(par,j), gp*512 + h*128 + par*64 + i] = e' (raw logits);
            # inactive cells stay NEG_BIG forever -> exp gives 0 every layer
            etT_w = wpool.tile([128, 16 * HID], F16, name=_nm("etw"), tag="etw")
            nc.gpsimd.memset(etT_w[:], NEG_BIG)
            aE_w = wpool.tile([128, 16 * HID], F16, name=_nm("aew"), tag="aew")
            for l in range(L):
                xrTb = [smpool.tile([128, G * 64], F16, name=_nm("xrTb"), tag=f"xrTb{m}",
                                    bufs=1) for m in range(2)]
                xlTb = [smpool.tile([128, G * 64], F16, name=_nm("xlTb"), tag=f"xlTb{m}",
                                    bufs=1) for m in range(2)]
                xlTn = [smpool.tile([128, G * 64], F16, name=_nm("xlTn"), tag=f"xlTn{m}",
                                    bufs=1) for m in range(2)]
                xlOb = [smpool.tile([128, G * 64], F16, name=_nm("xlOb"), tag=f"xlOb{m}",
                                    bufs=1) for m in range(2)]
                for m in range(2):
                    for cb in range(2):
                        pp = ppool.tile([128, 512], F32, name=_nm("pp"), tag="pps", bufs=1)
                        for k in range(2):
                            nc.tensor.matmul(pp[:], wl[l][k][:, m * 128:(m + 1) * 128],
                                             hT[k][:, cb * 512:(cb + 1) * 512],
                                             start=(k == 0), stop=(k == 1))
                        sl_ = (slice(None), slice(cb * 512, (cb + 1) * 512))
                        bcol = slice(l * 2 + m, l * 2 + m + 1)
                        nc.scalar.activation(xlTb[m][sl_], pp[:], AF.Identity,
                                             bias=blT[:, bcol])
                        nc.scalar.activation(xlTn[m][sl_], pp[:], AF.Identity,
                                             bias=nblT[:, bcol], scale=-1.0)
                        nc.scalar.activation(xlOb[m][sl_], pp[:], AF.Identity,
                                             bias=oblT[:, bcol])
                    for cb in range(2):
                        pp = ppool.tile([128, 512], F32, name=_nm("pp"), tag="pps", bufs=1)
                        for k in range(2):
                            nc.tensor.matmul(pp[:], wr[l][k][:, m * 128:(m + 1) * 128],
                                             hT[k][:, cb * 512:(cb + 1) * 512],
                                             start=(k == 0), stop=(k == 1))
                        nc.scalar.activation(
                            xrTb[m][:, cb * 512:(cb + 1) * 512], pp[:], AF.Identity,
                            bias=brT[:, l * 2 + m:l * 2 + m + 1])

                # xl node-major (with bl+ob bias) + per-gp ones block for Z
                xn0 = smpool.tile([128, 8 * 320], F16, name=_nm("xn"), tag="xn0",
                                  bufs=1)
                nc.vector.memset(fd(xn0[0:128, 256:257], (320, 8), (1, 64)), 1.0)
                for gp in range(8):
                    for hp in range(2):
                        dmae().dma_start_transpose(
                            xn0[:, gp * 320 + hp * 128:gp * 320 + hp * 128 + 128],
                            xlOb[hp][:, gp * 128:(gp + 1) * 128])

                # ---- attention ----
                for gp in range(8):
                    m = mh[2 * gp]
                    mm = m * m
                    nch, ipc = _chunking(m)
                    w = ipc * m
                    # pax[32*s+t, j] = (att_(hp,t) . xl_par)_j, s = 2*hp+par
                    # (stationary padded to 32 cols so full psum rows init)
                    pax = ppool.tile([128, 64], F32, name=_nm("pax"), tag="paxps",
                                     bufs=1)
                    for hp in range(2):
                        for par in range(2):
                            s = 2 * hp + par
                            nc.tensor.matmul(
                                pax[32 * s:32 * s + 32, 0:64],
                                att10[:, (l * 2 + hp) * 32:(l * 2 + hp) * 32 + 32],
                                xlTb[hp][:, (gp * 2 + par) * 64:(gp * 2 + par) * 64 + 64],
                                start=True, stop=True,
                                tile_position=(0, 32 * s))
                    waxl = smpool.tile([128, 64], F32, name=_nm("waxl"), tag="waxl",
                                       bufs=2)
                    nc.vector.tensor_tensor(waxl[:], pax[:],
                                            maskj[:, gp * 64:(gp + 1) * 64],
                                            op=ALU.add)

                    # sl tiles per (hp): cols par*mm + j*m + i  (j-major!)
                    slts = []
                    for hp in range(2):
                        slt = slpool.tile([128, 2 * mm], F16, name=_nm("sl"), tag="sl",
                                          bufs=1 if big_m else 2,
                                          padded_shape=[128, 2 * mmax * mmax])
                        for par in range(2):
                            g = gp * 2 + par
                            dst = fd(slt[:, par * mm:par * mm + 1], (m, m), (1, m))
                            xr_sl = xrTb[hp][:, g * 64:g * 64 + 1]
                            xl_sl = xlTn[hp][:, g * 64:g * 64 + 1]
                            nc.vector.tensor_tensor(dst, fd(xl_sl, (1, m), (0, m)),
                                                    fd(xr_sl, (0, m), (1, m)), op=ALU.max)
                        slts.append(slt)
                    # e matmuls: 4 streams share psum rows 32*s+t; one STT
                    # per 2-bank group evacuates all 4 streams at once.
                    # e_sb rows 32*s+t, cols i*m + j
                    e_sb = slpool.tile([128, mm], F16, name=_nm("esb"), tag="esb",
                                       bufs=1 if big_m else 2,
                                       padded_shape=[128, mmax * mmax])
                    for grp in range(nch // 2):
                        pe = ppool.tile([128, 1024], F32, name=_nm("pe"),
                                        tag="eps", bufs=2)
                        for b2 in range(2):
                            ci = grp * 2 + b2
                            for hp in range(2):
                                for par in range(2):
                                    s = 2 * hp + par
                                    nc.tensor.matmul(
                                        pe[32 * s:32 * s + 32, b2 * 512:b2 * 512 + w],
                                        att08[:, (l * 2 + hp) * 32:(l * 2 + hp) * 32 + 32],
                                        slts[hp][:, par * mm + ci * w:
                                                 par * mm + (ci + 1) * w],
                                        start=True, stop=True,
                                        tile_position=(0, 32 * s))
                        dst = fd(e_sb[0:128, grp * 2 * w:grp * 2 * w + 1],
                                 (1, 2 * w))
                        src0 = fd(pe[0:128, 0:1], (512, 2), (1, w))
                        src1 = fd(waxl[0:128, grp * 2 * ipc:grp * 2 * ipc + 1],
                                  (ipc, 2), (1, ipc), (0, m))
                        nc.vector.scalar_tensor_tensor(
                            dst, src0, 1.0, src1, op0=ALU.mult, op1=ALU.add)
                    # scatter transposed: e_sb row (s,t), cols (j,i) -> etT
                    for hp in range(2):
                        for par in range(2):
                            s = 2 * hp + par
                            for t in range(2):
                                src = fd(e_sb[32 * s + t:32 * s + t + 1, 0:1],
                                         (m, m), (1, m))
                                cb0 = gp * 512 + (2 * hp + t) * 128 + par * 64
                                db = etT_w[par * 64:par * 64 + m, cb0:cb0 + 1]
                                dstp = fd(db, (1, m))
                                dmae().dma_start(dstp, src)
                    # exp for this gp's slab (raw etT preserved)
                    nc.scalar.activation(aE_w[:, gp * 512:(gp + 1) * 512],
                                         etT_w[:, gp * 512:(gp + 1) * 512],
                                         AF.Exp, bias=nbias[:])

                # ---- attention out (node-major) + Z via ones column ----
                z_w = smpool.tile([128, 32], F32, name=_nm("zw"), tag="zw", bufs=2)
                gn_w = wpool.tile([128, 8 * HID], F32, name=_nm("gnw"), tag="gnw")
                for gp in range(8):
                    po = ppool.tile([128, 512], F32, name=_nm("po"), tag="ops", bufs=2)
                    for h_g in range(4):
                        mov = fd(xn0[0:128, gp * 320 + h_g * 64:gp * 320 + h_g * 64 + 1],
                                 (256 - h_g * 64, 2), (1, 64))
                        nc.tensor.matmul(
                            po[:, h_g * 128:h_g * 128 + 128],
                            aE_w[:, (gp * 4 + h_g) * 128:(gp * 4 + h_g) * 128 + 128],
                            mov, start=True, stop=True)
                    nc.scalar.activation(z_w[:, gp * 4:gp * 4 + 4],
                                         fd(po[0:128, 64:65], (128, 4)),
                                         AF.Identity, bias=0.0)
                    nc.scalar.activation(gn_w[:, gp * HID:(gp + 1) * HID],
                                         fd(po[0:128, 0:1], (128, 4), (1, 64)),
                                         AF.Identity, bias=0.0)
                rz_w = smpool.tile([128, 32], F32, name=_nm("rzw"), tag="rzw", bufs=2)
                nc.vector.reciprocal(rz_w[:], z_w[:])

                # ---- normalize by Z + ELU (fp16) ----
                gn16 = wpool.tile([128, 8 * HID], F16, name=_nm("gn16"), tag="gn16")
                nc.vector.tensor_tensor(gn16[:], gn_w[:],
                                        fd(rz_w[0:128, 0:1], (4, 8), (1, 4), (0, 64)),
                                        op=ALU.mult)
                tmin = wpool.tile([128, 8 * HID], F16, name=_nm("tmin"), tag="tmin")
                nc.vector.tensor_scalar(tmin[:], gn16[:], 0.0, None, op0=ALU.min)
                nc.scalar.activation(tmin[:], tmin[:], AF.Exp)
                nc.vector.tensor_scalar(gn16[:], gn16[:], 0.0, None, op0=ALU.max)
                nc.vector.scalar_tensor_tensor(gn16[:], gn16[:], -1.0, tmin[:],
                                               op0=ALU.add, op1=ALU.add)

                # ---- LayerNorm (Act-centered) + residual + mask ----
                sum_w = smpool.tile([128, 8], F32, name=_nm("sumw"), tag="sumw", bufs=2)
                nc.vector.tensor_reduce(sum_w[:],
                                        fd(gn16[0:128, 0:1], (HID, 8), (1, HID)),
                                        axis=mybir.AxisListType.X, op=ALU.add)
                mu_w = smpool.tile([128, 8], F32, name=_nm("muw"), tag="muw", bufs=2)
                nc.vector.tensor_scalar(mu_w[:], sum_w[:], 1.0 / HID, None, op0=ALU.mult)
                sq16 = wpool.tile([128, 8 * HID], F16, name=_nm("sq16"), tag="tmin")
                nc.scalar.activation(sq16[:], gn16[:], AF.Square)
                vs_w = smpool.tile([128, 8], F32, name=_nm("vsw"), tag="vsw", bufs=2)
                nc.vector.tensor_reduce(vs_w[:],
                                        fd(sq16[0:128, 0:1], (HID, 8), (1, HID)),
                                        axis=mybir.AxisListType.X, op=ALU.add)
                musq = smpool.tile([128, 8], F32, name=_nm("musq"), tag="musq", bufs=2)
                nc.gpsimd.tensor_tensor(musq[:], mu_w[:], mu_w[:], op=ALU.mult)
                var_w = smpool.tile([128, 8], F32, name=_nm("varw"), tag="varw", bufs=2)
                nc.vector.scalar_tensor_tensor(var_w[:], vs_w[:], 1.0 / HID, musq[:],
                                               op0=ALU.mult, op1=ALU.subtract)
                nc.scalar.activation(var_w[:], var_w[:], AF.Sqrt, bias=epsb[:])
                rstd_w = smpool.tile([128, 8], F32, name=_nm("rstdw"), tag="rstdw",
                                     bufs=2)
                nc.vector.reciprocal(rstd_w[:], var_w[:])
                nmr = smpool.tile([128, 8], F32, name=_nm("nmr"), tag="nmr", bufs=2)
                nc.vector.scalar_tensor_tensor(nmr[:], mu_w[:], -1.0, rstd_w[:],
                                               op0=ALU.mult, op1=ALU.mult)
                for gp in range(8):
                    nc.scalar.activation(gn16[:, gp * HID:(gp + 1) * HID],
                                         gn16[:, gp * HID:(gp + 1) * HID],
                                         AF.Identity, bias=nmr[:, gp:gp + 1],
                                         scale=rstd_w[:, gp:gp + 1])
                nc.vector.tensor_tensor(gn16[:], gn16[:],
                                        fd(gam[l][0:128, 0:1], (0, 8), (1, HID)),
                                        op=ALU.mult)
                nc.vector.tensor_tensor(gn16[:], gn16[:],
                                        fd(bet[l][0:128, 0:1], (0, 8), (1, HID)),
                                        op=ALU.add)
                hn_w = smpool.tile([128, 8 * HID], F16, name=_nm("hn"), tag="hnode",
                                   bufs=2)
                nc.vector.tensor_tensor(hn_w[:], gn16[:], h_node_w[:], op=ALU.add)
                nc.vector.tensor_tensor(hn_w[:], hn_w[:],
                                        fd(mvec[0:128, 0:1], (1, 8), (0, HID)),
                                        op=ALU.mult)
                h_node_w = hn_w

                # ---- next-layer hT via DMA transpose ----
                if l + 1 < L:
                    hT = [smpool.tile([128, G * 64], F16, name=_nm("hT"), tag=f"hT{m}",
                                      bufs=1) for m in range(2)]
                    for gp in range(8):
                        for m in range(2):
                            dmae().dma_start_transpose(
                                hT[m][:, gp * 128:(gp + 1) * 128],
                                hn_w[:, gp * HID + m * 128:gp * HID + m * 128 + 128])

            # ---------- output DMA ----------
            for par in range(2):
                src = fd(h_node_w[par * 64:par * 64 + 64, 0:1], (HID, 8), (1, HID))
                dst_sl = out_d[par * 64:par * 64 + 1, :]
                dst = bass.AP(dst_sl.tensor, dst_sl.offset,
                              [[HID, 64], [2 * 64 * HID, 8], [1, HID]])
                nc.sync.dma_start(dst, src)

    nc.finalize()
    return nc


_CACHE = {}

def _get_nc(mh):
    mh = tuple(mh)
    if mh not in _CACHE:
        _CACHE[mh] = build_nc(mh)
    return _CACHE[mh]


def _host_prep(x, person_mask, W_in, b_in, Wl, bl, Wr, br, att, out_bias, ln_scale, ln_bias):
    x = np.asarray(x, np.float32).reshape(BT, N, D_IN)
    m = np.asarray(person_mask).reshape(BT, N)
    W_in = np.asarray(W_in, np.float32)
    b_in = np.asarray(b_in, np.float32)
    Wl = np.asarray(Wl, np.float32)
    bl = np.asarray(bl, np.float32)
    Wr = np.asarray(Wr, np.float32)
    br = np.asarray(br, np.float32)
    att = np.asarray(att, np.float32)
    out_bias = np.asarray(out_bias, np.float32)
    ln_scale = np.asarray(ln_scale, np.float32)
    ln_bias = np.asarray(ln_bias, np.float32)

    # ---- pack active nodes; stripe sorted graphs across cores ----
    n_g = m.sum(-1).astype(np.int64)
    order = np.argsort(-n_g, kind="stable")
    idxs = [np.nonzero(m[g])[0] for g in range(BT)]
    mh = []
    for s in range(G):
        n_top = n_g[order[s * NCORES]]
        mh.append(max(8, int(-(-int(n_top) // 8) * 8)))
    for k in range(0, G, 2):
        mh[k + 1] = mh[k]
    mh = tuple(min(64, v) for v in mh)

    binT = np.zeros((128, 2), np.float32)
    for mm in range(2):
        binT[:, mm] = b_in[mm * 128:(mm + 1) * 128]
    blT = np.zeros((128, 2 * L), np.float32)
    brT = np.zeros((128, 2 * L), np.float32)
    oblT = np.zeros((128, 2 * L), np.float32)
    for l in range(L):
        for mm in range(2):
            blT[:, l * 2 + mm] = bl[l, mm * 128:(mm + 1) * 128]
            brT[:, l * 2 + mm] = br[l, mm * 128:(mm + 1) * 128]
            oblT[:, l * 2 + mm] = (bl[l] + out_bias[l])[mm * 128:(mm + 1) * 128]
    attBD = np.zeros((128, 32 * 2 * L), np.float32)
    for l in range(L):
        for hp in range(2):
            for t in range(2):
                attBD[t * 64:(t + 1) * 64, (l * 2 + hp) * 32 + t] = att[l, 2 * hp + t]
    gam_f = np.repeat(ln_scale[:, None, :], 128, 1).astype(np.float16).copy()
    bet_f = np.repeat(ln_bias[:, None, :], 128, 1).astype(np.float16).copy()
    W16 = np.ascontiguousarray(W_in.astype(np.float16))
    Wl16 = np.ascontiguousarray(Wl.astype(np.float16))
    Wr16 = np.ascontiguousarray(Wr.astype(np.float16))

    in_maps = []
    for c in range(NCORES):
        xg = np.zeros((G * 64, D_IN), np.float16)
        maskj_w = np.full((128, 8, 64), NEG_BIG, np.float32)
        mvec_w = np.zeros((128, 8), np.float16)
        for s in range(G):
            gg = order[s * NCORES + c]
            n = int(n_g[gg])
            gp, par = s // 2, s % 2
            if n > 0:
                xg[s * 64:s * 64 + n] = x[gg][idxs[gg]]
                for hp in range(2):
                    srow = 32 * (2 * hp + par)
                    maskj_w[srow:srow + 32, gp, 0:n] = 0.0
                mvec_w[par * 64 + np.arange(n), gp] = 1.0
            else:
                for hp in range(2):
                    srow = 32 * (2 * hp + par)
                    maskj_w[srow:srow + 32, gp, 0] = 0.0
        xgT = np.ascontiguousarray(xg.T)
        in_maps.append({
            "xT": xgT, "w_in": W16, "wl": Wl16, "wr": Wr16,
            "binT": binT, "blT": blT, "nblT": -blT, "oblT": oblT, "brT": brT,
            "attBD": attBD, "gam_f": gam_f, "bet_f": bet_f,
            "maskj_w": maskj_w.reshape(128, 8 * 64), "mvec_w": mvec_w,
        })
    return in_maps, x, m, W_in, b_in, order, idxs, n_g, mh


def kernel(**inputs) -> np.ndarray:
    in_maps, x, m, W_in, b_in, order, idxs, n_g, mh = _host_prep(**inputs)
    nc = _get_nc(mh)
    res = bass_utils.run_bass_kernel_spmd(nc, in_maps, core_ids=list(range(NCORES)))
    out = np.zeros((BT, N, HID), np.float32)
    for c in range(NCORES):
        dev = res.results[c]["out"].astype(np.float32).reshape(G, 64, HID)
        for s in range(G):
            gg = order[s * NCORES + c]
            n = int(n_g[gg])
            if n > 0:
                out[gg][idxs[gg]] = dev[s, :n]
    keep = n_g > 1
    if not keep.all():
        for g in np.nonzero(~keep)[0]:
            out[g] = x[g] @ W_in + b_in
    return out.reshape(B, T, N, HID)



# revision 36
# speedup vs baseline: 1.7078x; 1.7078x over previous
"""GATv2Stack Trainium2 kernel (8-core data-parallel over graphs), v3.

bt=128 graphs of N=64 nodes, 16 graphs/core. See reference.py.
  h = x @ W_in + b_in
  2x: xl=h@Wl+bl; xr=h@Wr+br; e=att.lrelu(xr_i+xl_j); a=softmax_j(e+mask)
      g = a@(h@Wl) + (out_bias+bl); g=ELU(g); g=LN(g); h=g+h
  out = where(keep_graph, h, x@W_in+b_in)

v3 design (from v2 trace: Scalar 58%, Vector 57%, DMA queue time ~165us):
  - w-factorization: exp(e'-4) = E_ij * w_j with
      E = exp(0.8*att.max(-xl_j, xr_i) - 2)   [fused into Act psum evac]
      w_j = exp((att.xl)_j + mask_j - 2)      [tiny Act exp of flipped pax]
    attention-out moving operand = w*xlOb (+w cols for Z), so the DVE
    scatter-add (STT) and separate exp pass are deleted entirely.
  - e-scatter DMAs grouped over equal-m gp runs: one DMA per
    (group, head, par, t) instead of per (gp, ...): ~64 -> ~24-32/layer;
    ALL DMAs issued on sync queue only (scalar SEQ freed for Act work).
  - all XBAR DMA transposes (xn0, h_node, hT) -> PE transposes + evacs
  - pairwise-MAX (dominant DVE op) split DVE/GpSimd ~4:3
  - rz (1/Z) folded into gn psum evac as per-partition Act scale
  - ELU's -1 dropped (LN-invariant); sum(x^2) via Act accum_out
Per-core layouts (G=16 graphs, gp pair idx, par=g%2):
  hT[m]     [128,1024] f16  [m*128+c, g*64+node]
  h_node    [128,2048] f16  [par*64+node, gp*256+ch]
  xlTn/xrTb/xlOb[hp] [128,1024] f16 (t,c) x (g,node)
  sl (gp,hp) [128,2*m*m]  f16 cols par*m*m + j*m + i
  e_all      [128, sum(mm)] f16 rows {32s+t}, cols eoff[gp]+par*... E vals
  aE_w      [128,2048] f16  [par*64+j, gp*256 + h*64?? no: gp*512/2..]
            actually [par*64+j, gp*512 + h*128 + par*64 + i] f16 = E
  xn0       [128, 8*320+8] f16 [par*64+node, gp*320 + hp*128 + t*64 + c],
            cols gp*320+256..260 = w_j per head
"""
import sys
sys.path.insert(0, '/opt/trn_rl_repo')
import numpy as np

import concourse.bass as bass
import concourse.mybir as mybir
from concourse import bass_utils, bacc
from concourse.tile import TileContext

dt = mybir.dt
F32, F16 = dt.float32, dt.float16
AF = mybir.ActivationFunctionType
ALU = mybir.AluOpType

B, T, N, D_IN = 2, 64, 64, 512
HID, L, H, C = 256, 2, 4, 64
BT = B * T
G = 16
NCORES = 8
LN_EPS = 1e-5
NEG_BIG = -30000.0
WB = 2.0  # bias split: E=exp(0.8*attmax-2), w=exp(attxl+mask-2)

_n = [0]
def _nm(p="t"):
    _n[0] += 1
    return f"{p}{_n[0]}"


def fd(ap, *dims):
    """Keep partition dim + offset of (sliced) AP, replace free dims."""
    return bass.AP(ap.tensor, ap.offset, [list(ap.ap[0])] + [[s, c] for (s, c) in dims])


def _chunking(m):
    """Uniform i-chunks: smallest even nch with (m/nch)*m <= 512."""
    nch = 2
    while (m // nch) * m > 512 or m % nch != 0:
        nch += 2
    return nch, m // nch


def build_nc(mh=(64,) * G):
    nc = bacc.Bacc("TRN2", target_bir_lowering=False, debug=False,
                   enable_asserts=False, num_devices=1)

    def din(name, shape, dtp=F16):
        return nc.dram_tensor(name, list(shape), dtp, kind="ExternalInput").ap()

    # merged inputs: few big DMAs instead of ~34 small ones
    xT_d    = din("xTm", [128, 4 * G * 64])          # 4 d-chunks side by side
    win_d   = din("winm", [128, 4 * HID])            # 4 d-chunks
    wlr_d   = din("wlrm", [128, 8 * HID])            # wl(l,k) 4x256, wr 4x256
    cf32_d  = din("cf32", [128, 150], F32)  # binT2 nblT4 oblT4 brT4 att128 attN8
    cf16_d  = din("cf16", [128, 2 * HID + 2 * HID + 128])  # gam, bet, idn
    mbT_d   = din("mbT", [128, 8], F32)              # mask - WB, [par*64+j, gp]
    out_d   = nc.dram_tensor("out", [G * 64, HID], F16, kind="ExternalOutput").ap()

    # per-gp m and equal-m groups (consecutive)
    gpm = [mh[2 * gp] for gp in range(8)]
    groups = []
    s0 = 0
    for gp in range(1, 9):
        if gp == 8 or gpm[gp] != gpm[s0]:
            groups.append((s0, gp))
            s0 = gp
    # e_all col layout: per gp block of mm cols = j*m+i; par lives in the
    # psum/e_all ROW (32*(2hp+par)+t), not in a column offset.
    eoff = [0] * 9
    for gp in range(8):
        eoff[gp + 1] = eoff[gp] + gpm[gp] * gpm[gp]
    etot = eoff[8]

    # engine rotation for small psum->sbuf evacs
    evc = [0]
    def evace():
        # psum -> sbuf evacs: GpSimd cannot access PSUM on TRN2
        evc[0] += 1
        return lambda dst, src: nc.scalar.activation(dst, src, AF.Identity)
    def maxe():
        return nc.vector

    with TileContext(nc) as tc:
        with tc.tile_pool(name="const", bufs=1) as cpool, \
             tc.tile_pool(name="wide", bufs=1) as wpool, \
             tc.tile_pool(name="slp", bufs=1) as slpool, \
             tc.tile_pool(name="sm", bufs=2) as smpool, \
             tc.tile_pool(name="psum", bufs=1, space="PSUM") as ppool:

            def ctile(name, dram_ap, shape, dtp=F16, eng=None):
                t0 = cpool.tile(shape, dtp, name=_nm(name))
                (eng or nc.gpsimd).dma_start(t0[:], dram_ap)
                return t0

            winm = ctile("winm", win_d, [128, 4 * HID], eng=nc.sync)
            win = [winm[:, k * HID:(k + 1) * HID] for k in range(4)]
            wlrm = ctile("wlrm", wlr_d, [128, 8 * HID], eng=nc.scalar)
            wl = [[wlrm[:, (l * 2 + k) * HID:(l * 2 + k + 1) * HID]
                   for k in range(2)] for l in range(L)]
            wr = [[wlrm[:, 4 * HID + (l * 2 + k) * HID:
                        4 * HID + (l * 2 + k + 1) * HID]
                   for k in range(2)] for l in range(L)]
            cf32 = ctile("cf32", cf32_d, [128, 150], F32, eng=nc.scalar)
            binT = cf32[:, 0:2]
            nblT = cf32[:, 2:6]
            oblT = cf32[:, 6:10]
            brT = cf32[:, 10:14]
            att10 = cpool.tile([128, 32 * 2 * L], F16, name=_nm("att10"))
            nc.vector.tensor_copy(att10[:], cf32[:, 14:142])
            attN = cpool.tile([128, 4 * L], F16, name=_nm("attN"))
            nc.vector.tensor_copy(attN[:], cf32[:, 142:150])
            cf16 = ctile("cf16", cf16_d, [128, 4 * HID + 128], eng=nc.scalar)
            gam = [cf16[:, l * HID:(l + 1) * HID] for l in range(L)]
            bet = [cf16[:, 2 * HID + l * HID:2 * HID + (l + 1) * HID]
                   for l in range(L)]
            idn = cf16[:, 4 * HID:4 * HID + 128]
            mbT = ctile("mbT", mbT_d, [128, 8], F32)
            epsb = cpool.tile([128, 1], F32, name=_nm("epsb"))
            nc.vector.memset(epsb[:], LN_EPS)
            ebias = cpool.tile([128, 1], F32, name=_nm("ebias"))
            nc.vector.memset(ebias[:], -WB)

            # aE: exp'd logits; cross-par / pad cells must be EXACTLY 0
            # (they sit inside attention-out stationary slabs).
            aE_w = wpool.tile([128, 16 * HID], F16, name=_nm("aew"), tag="aew")
            nc.gpsimd.memset(aE_w[:, 0:2048], 0.0)
            nc.vector.memset(aE_w[:, 2048:4096], 0.0)

            # ---------- input: load xT (d-major), project ----------
            hT = [smpool.tile([128, G * 64], F16, name=_nm("hT"), tag=f"hT{m}", bufs=1)
                  for m in range(2)]
            with tc.tile_pool(name="xtp", bufs=1) as xtpool:
                xTm = xtpool.tile([128, 4 * G * 64], F16, name=_nm("xT"))
                for hh in range(2):
                    nc.sync.dma_start(xTm[:, hh * 2048:(hh + 1) * 2048],
                                      xT_d[:, hh * 2048:(hh + 1) * 2048])
                xT = [xTm[:, k * 1024:(k + 1) * 1024] for k in range(4)]
                for m in range(2):
                    for cb in range(2):
                        ph = ppool.tile([128, 512], F32, name=_nm("ph"), tag="pps", bufs=2)
                        for k in range(4):
                            nc.tensor.matmul(ph[:], win[k][:, m * 128:(m + 1) * 128],
                                             xT[k][:, cb * 512:(cb + 1) * 512],
                                             start=(k == 0), stop=(k == 3))
                        nc.scalar.activation(hT[m][:, cb * 512:(cb + 1) * 512], ph[:],
                                             AF.Identity, bias=binT[:, m:m + 1])

            # h_node via PE transposes
            h_node_w = smpool.tile([128, 8 * HID], F16, name=_nm("hnode"), tag="hnode",
                                   bufs=2)
            for gp in range(8):
                for m in range(2):
                    tp = ppool.tile([128, 128], F16, name=_nm("tp"), tag="tp", bufs=2)
                    nc.tensor.transpose(tp[:], hT[m][:, gp * 128:(gp + 1) * 128],
                                        idn[:])
                    evace()(
                        h_node_w[:, gp * HID + m * 128:gp * HID + m * 128 + 128],
                        tp[:])

            # ---------- layers ----------
            for l in range(L):
                xrTb = [smpool.tile([128, G * 64], F16, name=_nm("xrTb"), tag=f"xrTb{m}",
                                    bufs=1) for m in range(2)]
                xlTn = [smpool.tile([128, G * 64], F16, name=_nm("xlTn"), tag=f"xlTn{m}",
                                    bufs=1) for m in range(2)]
                xlOb = [smpool.tile([128, G * 64], F16, name=_nm("xlOb"), tag=f"xlOb{m}",
                                    bufs=1) for m in range(2)]
                # cb-major: all of chunk cb=0 (gps 0-3) finishes first so the
                # first gps' sl-MAX starts as early as possible
                for cb in range(2):
                    for m in range(2):
                        pp = ppool.tile([128, 512], F32, name=_nm("pp"), tag="pps", bufs=2)
                        for k in range(2):
                            nc.tensor.matmul(pp[:], wl[l][k][:, m * 128:(m + 1) * 128],
                                             hT[k][:, cb * 512:(cb + 1) * 512],
                                             start=(k == 0), stop=(k == 1))
                        sl_ = (slice(None), slice(cb * 512, (cb + 1) * 512))
                        bcol = slice(l * 2 + m, l * 2 + m + 1)
                        nc.scalar.activation(xlTn[m][sl_], pp[:], AF.Identity,
                                             bias=nblT[:, bcol], scale=-1.0)
                        nc.scalar.activation(xlOb[m][sl_], pp[:], AF.Identity,
                                             bias=oblT[:, bcol])
                    for m in range(2):
                        pp = ppool.tile([128, 512], F32, name=_nm("pp"), tag="pps", bufs=2)
                        for k in range(2):
                            nc.tensor.matmul(pp[:], wr[l][k][:, m * 128:(m + 1) * 128],
                                             hT[k][:, cb * 512:(cb + 1) * 512],
                                             start=(k == 0), stop=(k == 1))
                        nc.scalar.activation(
                            xrTb[m][:, cb * 512:(cb + 1) * 512], pp[:], AF.Identity,
                            bias=brT[:, l * 2 + m:l * 2 + m + 1])

                # ---- w_j = exp(att.xl + mask - WB) via flipped pax ----
                wT = smpool.tile([128, 32], F32, name=_nm("wT"), tag="wT", bufs=2)
                for gp in range(8):
                    paxp = ppool.tile([128, 512], F32, name=_nm("paxp"), tag="ops",
                                      bufs=2)
                    for par in range(2):
                        g = gp * 2 + par
                        for hp in range(2):
                            nc.tensor.matmul(
                                paxp[par * 64:par * 64 + 64, hp * 2:hp * 2 + 2],
                                xlTn[hp][:, g * 64:g * 64 + 64],
                                attN[:, l * 4 + hp * 2:l * 4 + hp * 2 + 2],
                                start=True, stop=True,
                                tile_position=(0, 64 * par))
                    nc.scalar.activation(wT[:, gp * 4:gp * 4 + 4], paxp[:, 0:4],
                                         AF.Exp, bias=mbT[:, gp:gp + 1])

                # ---- xn0 = w * xlOb node-major (PE transpose) + w cols ----
                xn0 = smpool.tile([128, 8 * 320 + 64], F16, name=_nm("xn"), tag="xn0",
                                  bufs=1)
                for gp in range(8):
                    for hp in range(2):
                        tp = ppool.tile([128, 128], F16, name=_nm("tp"), tag="tp",
                                        bufs=2)
                        nc.tensor.transpose(tp[:], xlOb[hp][:, gp * 128:(gp + 1) * 128],
                                            idn[:])
                        evace()(xn0[:, gp * 320 + hp * 128:gp * 320 + hp * 128 + 128],
                                tp[:])
                    evace()(xn0[:, gp * 320 + 256:gp * 320 + 260],
                            wT[:, gp * 4:gp * 4 + 4])
                # scale xlOb rows by w_j: one wide op, per (gp, h) 64-col block
                nc.vector.tensor_tensor(
                    fd(xn0[0:128, 0:1], (320, 8), (64, 4), (1, 64)),
                    fd(xn0[0:128, 0:1], (320, 8), (64, 4), (1, 64)),
                    fd(wT[0:128, 0:1], (4, 8), (1, 4), (0, 64)), op=ALU.mult)

                # ---- attention: E = exp(0.8*att.max - WB) ----
                e_all = wpool.tile([128, etot], F16, name=_nm("eall"), tag="eall")
                for gp in range(8):
                    m = gpm[gp]
                    mm = m * m
                    nch, ipc = _chunking(m)
                    w = ipc * m
                    # sl tiles per (hp): cols par*mm + j*m + i  (j-major)
                    slts = []
                    for hp in range(2):
                        slt = slpool.tile([128, 2 * mm], F16, name=_nm("sl"), tag="sl",
                                          bufs=3, padded_shape=[128, 2 * 64 * 64])
                        for par in range(2):
                            g = gp * 2 + par
                            dst = fd(slt[:, par * mm:par * mm + 1], (m, m), (1, m))
                            xr_sl = xrTb[hp][:, g * 64:g * 64 + 1]
                            xl_sl = xlTn[hp][:, g * 64:g * 64 + 1]
                            maxe().tensor_tensor(dst, fd(xl_sl, (1, m), (0, m)),
                                                 fd(xr_sl, (0, m), (1, m)), op=ALU.max)
                        slts.append(slt)
                    # e matmuls: 4 streams share psum rows 32*s+t; Act evac
                    # fuses exp: E = exp(0.8*pe - WB)
                    for ci in range(nch):
                        pe = ppool.tile([128, 512], F32, name=_nm("pe"),
                                        tag="eps", bufs=2)
                        for hp in range(2):
                            for par in range(2):
                                s = 2 * hp + par
                                nc.tensor.matmul(
                                    pe[32 * s:32 * s + 2, 0:w],
                                    att10[:, (l * 2 + hp) * 32:(l * 2 + hp) * 32 + 2],
                                    slts[hp][:, par * mm + ci * w:
                                             par * mm + (ci + 1) * w],
                                    start=True, stop=True,
                                    tile_position=(0, 32 * s))
                        nc.scalar.activation(
                            e_all[:, eoff[gp] + ci * w:eoff[gp] + (ci + 1) * w],
                            pe[:, 0:w], AF.Exp, bias=ebias[:], scale=0.8)

                # ---- scatter: e_all -> aE_w (per gp; DMA APs cap at 3 dims
                # so the equal-m group merge is not expressible SBUF->SBUF) --
                for gp in range(8):
                    m = gpm[gp]
                    mm = m * m
                    for hp in range(2):
                        for par in range(2):
                            s = 2 * hp + par
                            for t in range(2):
                                src = fd(e_all[32 * s + t:32 * s + t + 1,
                                               eoff[gp]:eoff[gp] + 1],
                                         (m, m), (1, m))
                                cb0 = gp * 512 + (2 * hp + t) * 128 + par * 64
                                db = aE_w[par * 64:par * 64 + m, cb0:cb0 + 1]
                                dstp = fd(db, (1, m))
                                (nc.sync if (hp + par + t + gp) % 2 else
                                 nc.gpsimd).dma_start(dstp, src)

                # ---- attention out (node-major) + Z via w cols ----
                gn16 = wpool.tile([128, 8 * HID], F16, name=_nm("gn16"), tag="gn16")
                rz_w = smpool.tile([128, 32], F32, name=_nm("rzw"), tag="rzw", bufs=2)
                tmin = wpool.tile([128, 8 * HID], F16, name=_nm("tmin"), tag="tmin")
                sum_w = smpool.tile([128, 8], F32, name=_nm("sumw"), tag="sumw", bufs=2)
                vs_w = smpool.tile([128, 8], F32, name=_nm("vsw"), tag="vsw", bufs=2)
                sqs = smpool.tile([128, HID], F16, name=_nm("sqs"), tag="sqs", bufs=2)
                for gp in range(8):
                    po = ppool.tile([128, 512], F32, name=_nm("po"), tag="ops", bufs=2)
                    for h_g in range(4):
                        mov = fd(xn0[0:128, gp * 320 + h_g * 64:gp * 320 + h_g * 64 + 1],
                                 (256 - 63 * h_g, 2), (1, 64))
                        nc.tensor.matmul(
                            po[:, h_g * 128:h_g * 128 + 128],
                            aE_w[:, (gp * 4 + h_g) * 128:(gp * 4 + h_g) * 128 + 128],
                            mov, start=True, stop=True)
                    # clamp Z away from 0 (pad columns i>=m have Z=0); the
                    # clamped rz multiplies an exactly-0 numerator -> 0.
                    zsb = smpool.tile([128, 4], F32, name=_nm("zsb"), tag="zsb",
                                      bufs=2)
                    nc.vector.tensor_scalar(zsb[:], fd(po[0:128, 64:65], (128, 4)),
                                            1e-30, None, op0=ALU.max)
                    nc.vector.reciprocal(rz_w[:, gp * 4:gp * 4 + 4], zsb[:])
                    nc.scalar.activation(
                        gn16[:, gp * HID:gp * HID + HID],
                        fd(po[0:128, 0:1], (128, 4), (1, 64)),
                        AF.Identity)
                    # per-gp tail (pipelines with later gps' attention):
                    # rz mult, ELU (sans -1), LN sums via accum
                    gslc = fd(gn16[0:128, gp * HID:gp * HID + 1], (64, 4), (1, 64))
                    nc.vector.tensor_tensor(
                        gslc, gslc,
                        fd(rz_w[0:128, gp * 4:gp * 4 + 1], (1, 4), (0, 64)),
                        op=ALU.mult)
                    sl8 = slice(gp * HID, (gp + 1) * HID)
                    nc.vector.tensor_scalar(tmin[:, sl8], gn16[:, sl8], 0.0, None,
                                            op0=ALU.min)
                    nc.scalar.activation(tmin[:, sl8], tmin[:, sl8], AF.Exp)
                    nc.vector.scalar_tensor_tensor(
                        gn16[:, sl8], gn16[:, sl8], 0.0, tmin[:, sl8],
                        op0=ALU.max, op1=ALU.add, accum_out=sum_w[:, gp:gp + 1])
                    nc.scalar.activation(sqs[:], gn16[:, sl8], AF.Square,
                                         accum_out=vs_w[:, gp:gp + 1])

                # ---- LayerNorm stats (whole-layer; one Sqrt site/layer
                # keeps Act table swaps to one pair per layer) ----
                mu_w = smpool.tile([128, 8], F32, name=_nm("muw"), tag="muw", bufs=2)
                musq = smpool.tile([128, 8], F32, name=_nm("musq"), tag="musq", bufs=2)
                var_w = smpool.tile([128, 8], F32, name=_nm("varw"), tag="varw", bufs=2)
                rstd_w = smpool.tile([128, 8], F32, name=_nm("rstdw"), tag="rstdw",
                                     bufs=2)
                nmr = smpool.tile([128, 8], F32, name=_nm("nmr"), tag="nmr", bufs=2)
                nc.vector.tensor_scalar(mu_w[:], sum_w[:], 1.0 / HID, None,
                                        op0=ALU.mult)
                nc.vector.tensor_tensor(musq[:], mu_w[:], mu_w[:], op=ALU.mult)
                nc.vector.scalar_tensor_tensor(var_w[:], vs_w[:], 1.0 / HID,
                                               musq[:], op0=ALU.mult,
                                               op1=ALU.subtract)
                nc.scalar.activation(var_w[:], var_w[:], AF.Sqrt, bias=epsb[:])
                nc.vector.reciprocal(rstd_w[:], var_w[:])
                nc.vector.scalar_tensor_tensor(nmr[:], mu_w[:], -1.0, rstd_w[:],
                                               op0=ALU.mult, op1=ALU.mult)
                hn_w = smpool.tile([128, 8 * HID], F16, name=_nm("hn"), tag="hnode",
                                   bufs=2)
                if l + 1 < L:
                    hT = [smpool.tile([128, G * 64], F16, name=_nm("hT"), tag=f"hT{m}",
                                      bufs=1) for m in range(2)]
                for gp in range(8):
                    sl8 = slice(gp * HID, (gp + 1) * HID)
                    nc.scalar.activation(gn16[:, sl8], gn16[:, sl8],
                                         AF.Identity, bias=nmr[:, gp:gp + 1],
                                         scale=rstd_w[:, gp:gp + 1])
                    # contiguous per-gp gamma/beta (2x-eligible), then residual
                    nc.vector.tensor_tensor(gn16[:, sl8], gn16[:, sl8],
                                            gam[l][:, :], op=ALU.mult)
                    nc.vector.tensor_tensor(gn16[:, sl8], gn16[:, sl8],
                                            bet[l][:, :], op=ALU.add)
                    nc.vector.tensor_tensor(hn_w[:, sl8], gn16[:, sl8],
                                            h_node_w[:, sl8], op=ALU.add)
                    if l + 1 < L:
                        for m in range(2):
                            tp = ppool.tile([128, 128], F16, name=_nm("tp"), tag="tp",
                                            bufs=2)
                            nc.tensor.transpose(
                                tp[:],
                                hn_w[:, gp * HID + m * 128:gp * HID + m * 128 + 128],
                                idn[:])
                            evace()(hT[m][:, gp * 128:(gp + 1) * 128],
                                    tp[:])
                h_node_w = hn_w

            # ---------- output DMA ----------
            for par in range(2):
                src = fd(h_node_w[par * 64:par * 64 + 64, 0:1], (HID, 8), (1, HID))
                dst_sl = out_d[par * 64:par * 64 + 1, :]
                dst = bass.AP(dst_sl.tensor, dst_sl.offset,
                              [[HID, 64], [2 * 64 * HID, 8], [1, HID]])
                nc.sync.dma_start(dst, src)

    nc.finalize()
    return nc
